# revision 1
# baseline (speedup 1.0000x reference)
"""Trainium2 Bass kernel for nn_HaarDecomposer2D.

The reference module (diagonal Haar decompose + reconstruct, channel-summed)
is algebraically out[b,0,h,w] = 0.5 * sum_c x[b,c,h,w]:
the decompose/recon coefficient products telescope to 0.5 * identity per
2x2-block pixel position. Verified vs the jax reference at ~6e-8 rel err.

Strategy: pure data parallel over batch (16 images -> 2 per core x 8 cores).
Per core: stream [128, 4096] f32 chunks (2 MiB per DMA), sum the 3 channels
on DVE, scale by 0.5 on ACT, DMA back out. Memory-bound: 33.5 MB/core.
"""

import sys

for p in ("/opt/trn_rl_repo",):
    if p not in sys.path:
        sys.path.insert(0, p)

import numpy as np

import concourse.bacc as bacc
import concourse.mybir as mybir
import concourse.tile as tile
from concourse.bass_utils import run_bass_kernel_spmd

N_CORES = 8
B_FULL, C, H, W = 16, 3, 1024, 1024
NB = B_FULL // N_CORES  # batches per core
P = 128                 # SBUF partitions
F = 4096                # free-dim elems per chunk (16 KB/partition, 2 MiB/DMA)
NJ = (H * W) // (P * F) # chunks per image plane

_cache = {}


def _build():
    nc = bacc.Bacc("TRN2", target_bir_lowering=False, debug=False)
    x = nc.dram_tensor("x", [NB, C, NJ, P, F], mybir.dt.float32,
                       kind="ExternalInput")
    o = nc.dram_tensor("out", [NB, NJ, P, F], mybir.dt.float32,
                       kind="ExternalOutput")

    with tile.TileContext(nc) as tc:
        with tc.tile_pool(name="io", bufs=2) as pin, \
             tc.tile_pool(name="res", bufs=3) as pres:
            for b in range(NB):
                for j in range(NJ):
                    ct = pin.tile([P, C, F], mybir.dt.float32, tag="c")
                    nc.sync.dma_start(
                        out=ct[:, :, :],
                        in_=x[b, :, j, :, :].rearrange("c p f -> p c f"))
                    nc.vector.tensor_add(ct[:, 0, :], ct[:, 0, :], ct[:, 1, :])
                    nc.vector.tensor_add(ct[:, 0, :], ct[:, 0, :], ct[:, 2, :])
                    ot = pres.tile([P, F], mybir.dt.float32, tag="o")
                    nc.scalar.mul(ot[:, :], ct[:, 0, :], 0.5)
                    nc.sync.dma_start(out=o[b, j, :, :], in_=ot[:, :])
    nc.finalize()
    return nc


def kernel(x: np.ndarray) -> np.ndarray:
    assert x.shape == (B_FULL, C, H, W) and x.dtype == np.float32
    if "nc" not in _cache:
        _cache["nc"] = _build()
    nc = _cache["nc"]

    xs = np.ascontiguousarray(x).reshape(N_CORES, NB, C, NJ, P, F)
    in_maps = [{"x": xs[i]} for i in range(N_CORES)]
    res = run_bass_kernel_spmd(nc, in_maps, core_ids=list(range(N_CORES)))
    out = np.stack([r["out"] for r in res.results], axis=0)
    return out.reshape(B_FULL, 1, H, W)

